# revision 16
# baseline (speedup 1.0000x reference)
"""GAT CFG-encoder kernel for Trainium2 (8 NeuronCores, batch-parallel).

Reference math (per sample b, N=1024 nodes, D=256, H=4 heads, DH=64, 2 layers):
  node_feat = mean_l emb[nodes[n, l]]            (gather + mean over L=16)
  adj       = (cfg_adj | I)                       (self loops)
  per layer: Wh = h @ W[l]; src/dst = Wh . a; e = leaky_relu(src[n]+dst[m]);
             alpha = softmax_m(mask(e)); h' = alpha @ Wh; h = LN(h + concat_h h')

Device-side layout tricks:
  - everything attention-side is computed TRANSPOSED: eT[m, n] = dst[m] + src[n],
    built by one ACT pass (Lrelu with per-partition bias = dst) over a
    PE-broadcast src row; softmax denominator comes out of the h' matmul as an
    extra all-ones weight column, so no cross-partition reductions anywhere.
  - exp runs without max-subtraction (|e| is bounded by ~20 here) and the
    adjacency mask is applied multiplicatively after exp; masked logits would
    hit exp() underflow at -1e9 anyway, so results match jax softmax+where.
  - host pre-transposes the adjacency, pre-scales emb by 1/16 (the token mean),
    and pre-packs [W | W@a_src | W@a_dst] so the kernel sees 2 clean matmuls.
"""

import os
import numpy as np

B, N, L, D, H, NLAYERS, V = 8, 1024, 16, 256, 4, 2, 32000
DH = D // H
P = 128
NB = N // P  # node blocks per core

_CACHE = {}


def _build_program():
    import concourse.bacc as bacc
    import concourse.bass as bass
    import concourse.tile as tile
    from concourse import mybir
    from concourse.masks import make_identity

    f32 = mybir.dt.float32
    i32 = mybir.dt.int32
    AF = mybir.ActivationFunctionType
    OP = mybir.AluOpType
    AX = mybir.AxisListType

    nc = bacc.Bacc("TRN2", target_bir_lowering=False, debug=False)

    # ---- DRAM I/O (per-core views; host slices the batch) ----
    adjtf = nc.dram_tensor("adjtf", [N, N], f32, kind="ExternalInput").ap()
    gidx = nc.dram_tensor("gidx", [P, NB, L], i32, kind="ExternalInput").ap()
    embs = nc.dram_tensor("embs", [V, D], f32, kind="ExternalInput").ap()
    wl = nc.dram_tensor("wl", [NLAYERS, 2, P, D + 8], f32, kind="ExternalInput").ap()
    wsrcb = nc.dram_tensor("wsrcb", [NLAYERS, H, 2, P, P], f32, kind="ExternalInput").ap()
    lnw = nc.dram_tensor("lnw", [NLAYERS, 2, D], f32, kind="ExternalInput").ap()
    hout = nc.dram_tensor("hout", [N, D], f32, kind="ExternalOutput").ap()
    dbg = {}
    if os.environ.get("KERNEL_DEBUG_DUMP"):
        for nm, shp in [
            ("d_h0", [N, D]),
            ("d_srcdst", [N, 2 * H]),
            ("d_whall", [N, H * (DH + 1)]),
            ("d_alpha00", [P, N]),
            ("d_psb0", [P, N]),
            ("d_hcat0", [N, D]),
            ("d_mask0", [P, N]),
        ]:
            dbg[nm] = nc.dram_tensor(nm, shp, f32, kind="ExternalOutput").ap()

    from contextlib import ExitStack

    with tile.TileContext(nc) as tc, ExitStack() as ctx:
        res = ctx.enter_context(tc.tile_pool(name="res", bufs=1))
        work = ctx.enter_context(tc.tile_pool(name="work", bufs=3))
        small = ctx.enter_context(tc.tile_pool(name="small", bufs=4))
        alpha_pool = ctx.enter_context(tc.tile_pool(name="alpha", bufs=1))
        psum_sb = ctx.enter_context(tc.tile_pool(name="psum_sb", bufs=1, space="PSUM"))
        psum_hp = ctx.enter_context(tc.tile_pool(name="psum_hp", bufs=1, space="PSUM"))
        psum_t = ctx.enter_context(tc.tile_pool(name="psum_t", bufs=2, space="PSUM"))
        psum_w = ctx.enter_context(tc.tile_pool(name="psum_w", bufs=2, space="PSUM"))

        # ---- constants ----
        ident = res.tile([P, P], f32)
        make_identity(nc, ident)
        eps_t = res.tile([P, 1], f32)
        nc.vector.memset(eps_t, 1e-5)

        # ---- resident tensors ----
        maskT = [res.tile([P, N], f32, tag=f"maskT{m}", name=f"maskT{m}") for m in range(NB)]
        h0 = [res.tile([P, D], f32, tag=f"h0_{n}", name=f"h0_{n}") for n in range(NB)]
        hcat = [res.tile([P, D], f32, tag=f"hcat{n}", name=f"hcat{n}") for n in range(NB)]
        hT = [res.tile([P, N], f32, tag=f"hT{k}", name=f"hT{k}") for k in range(2)]
        whall = [res.tile([P, H * (DH + 1)], f32, tag=f"whall{m}", name=f"whall{m}") for m in range(NB)]
        srcdst = [res.tile([P, 2 * H], f32, tag=f"srcdst{m}", name=f"srcdst{m}") for m in range(NB)]
        wsb = [
            [
                [
                    res.tile([P, P], f32, tag=f"wsb{l}_{h}_{k}", name=f"wsb{l}_{h}_{k}")
                    for k in range(2)
                ]
                for h in range(H)
            ]
            for l in range(NLAYERS)
        ]
        wl_sb = [
            [res.tile([P, D + 8], f32, tag=f"wl{l}_{k}", name=f"wl{l}_{k}") for k in range(2)]
            for l in range(NLAYERS)
        ]
        ln_s = [res.tile([P, D], f32, tag=f"lns{l}", name=f"lns{l}") for l in range(NLAYERS)]
        ln_b = [res.tile([P, D], f32, tag=f"lnb{l}", name=f"lnb{l}") for l in range(NLAYERS)]
        gidx_t = res.tile([P, NB, L], i32)

        # ones column inside each whall head-group (softmax denominator trick)
        for m in range(NB):
            wv = whall[m][:].rearrange("p (h c) -> p h c", h=H)
            nc.vector.memset(wv[:, :, DH : DH + 1], 1.0)

        # ---- phase A: loads + gather ----
        nc.sync.dma_start(gidx_t[:], gidx)
        for l in range(NLAYERS):
            for k in range(2):
                nc.sync.dma_start(wl_sb[l][k][:], wl[l, k])
            for h in range(H):
                for k in range(2):
                    nc.sync.dma_start(wsb[l][h][k][:], wsrcb[l, h, k])
            # broadcast (1, D) across 128 partitions via DRE replication
            for dst_t, row in ((ln_s[l], lnw[l, 0]), (ln_b[l], lnw[l, 1])):
                bcast = bass.AP(
                    tensor=row.tensor, offset=row.offset, ap=[[0, P]] + list(row.ap)
                )
                nc.gpsimd.dma_start(out=dst_t[:], in_=bcast)
        for m in range(NB):
            nc.sync.dma_start(maskT[m][:], adjtf[m * P : (m + 1) * P, :])

        # Gather+mean: one indirect DMA per (block, token-slot); HW supports one
        # index per partition per call. compute_op=add accumulates the L token
        # embeddings (emb pre-scaled by 1/16 on host) straight into h0[nb].
        for nb in range(NB):
            nc.vector.memset(h0[nb][:], 0.0)
            for l in range(L):
                nc.gpsimd.indirect_dma_start(
                    out=h0[nb][:],
                    out_offset=None,
                    in_=embs,
                    in_offset=bass.IndirectOffsetOnAxis(
                        ap=gidx_t[:, nb, l : l + 1], axis=0
                    ),
                    compute_op=OP.add,
                )

        def build_hT():
            for nb in range(NB):
                for k in range(2):
                    pt = psum_t.tile([P, P], f32, tag="pt")
                    nc.tensor.transpose(pt[:], h0[nb][:, k * P : (k + 1) * P], ident[:])
                    nc.vector.tensor_copy(hT[k][:, nb * P : (nb + 1) * P], pt[:])

        build_hT()
        if dbg:
            for nb in range(NB):
                nc.sync.dma_start(dbg["d_h0"][nb * P : (nb + 1) * P, :], h0[nb][:])
            nc.sync.dma_start(dbg["d_mask0"][:], maskT[0][:])

        # ---- phase B: layers ----
        for l in range(NLAYERS):
            # Wh (all heads) + src/dst projections, per node block
            for m in range(NB):
                pw = psum_w.tile([P, D + 8], f32, tag="pw")
                for k in range(2):
                    nc.tensor.matmul(
                        pw[:],
                        hT[k][:, m * P : (m + 1) * P],
                        wl_sb[l][k][:],
                        start=(k == 0),
                        stop=(k == 1),
                    )
                for h in range(H):
                    nc.vector.tensor_copy(
                        whall[m][:, h * (DH + 1) : h * (DH + 1) + DH],
                        pw[:, h * DH : (h + 1) * DH],
                    )
                nc.vector.tensor_copy(srcdst[m][:], pw[:, D : D + 8])
                if dbg and l == 0:
                    nc.sync.dma_start(
                        dbg["d_srcdst"][m * P : (m + 1) * P, :], srcdst[m][:]
                    )
                    nc.sync.dma_start(
                        dbg["d_whall"][m * P : (m + 1) * P, :], whall[m][:]
                    )

            for h in range(H):
                # src broadcast: psb[p, n] = src[n] for every partition p
                psb = psum_sb.tile([P, N], f32, tag="psb")
                for c in range(2):
                    cs = slice(c * 512, (c + 1) * 512)
                    for k in range(2):
                        nc.tensor.matmul(
                            psb[:, cs],
                            wsb[l][h][k][:],
                            hT[k][:, cs],
                            start=(k == 0),
                            stop=(k == 1),
                        )
                # alphaT[m] = exp(leaky_relu(dst[m] + src[n])) * mask
                at = []
                for m in range(NB):
                    t1 = work.tile([P, N], f32, tag="t1")
                    nc.scalar.activation(
                        out=t1[:],
                        in_=psb[:],
                        func=AF.Prelu,
                        bias=srcdst[m][:, 2 * h + 1 : 2 * h + 2],
                        scale=1.0,
                        alpha=0.2,
                    )
                    t2 = work.tile([P, N], f32, tag="t2")
                    nc.scalar.activation(out=t2[:], in_=t1[:], func=AF.Exp)
                    a_t = alpha_pool.tile([P, N], f32, tag=f"a{m}", name=f"a{m}")
                    nc.vector.tensor_tensor(
                        out=a_t[:], in0=t2[:], in1=maskT[m][:], op=OP.mult
                    )
                    at.append(a_t)
                    if dbg and l == 0 and h == 0 and m == 0:
                        nc.sync.dma_start(dbg["d_alpha00"][:], a_t[:])
                        psb_sb = work.tile([P, N], f32, tag="t1")
                        nc.vector.tensor_copy(psb_sb[:], psb[:])
                        nc.sync.dma_start(dbg["d_psb0"][:], psb_sb[:])
                # h'T (+ colsum row 64) = sum_m whall[m].T @ alphaT[m]
                php = psum_hp.tile([DH + 1, N], f32, tag="php")
                for m in range(NB):
                    for c in range(2):
                        cs = slice(c * 512, (c + 1) * 512)
                        nc.tensor.matmul(
                            php[:, cs],
                            whall[m][:, h * (DH + 1) : (h + 1) * (DH + 1)],
                            at[m][:, cs],
                            start=(m == 0),
                            stop=(m == NB - 1),
                        )
                hp_sb = work.tile([DH + 1, N], f32, tag="hp")
                nc.vector.tensor_copy(hp_sb[:], php[:])
                # back to (node, feat) layout + normalize by colsum
                for nb in range(NB):
                    pt = psum_t.tile([P, DH + 1], f32, tag="pt")
                    nc.tensor.transpose(
                        pt[:], hp_sb[:, nb * P : (nb + 1) * P], ident[: DH + 1, : DH + 1]
                    )
                    rec = small.tile([P, 1], f32, tag="rec")
                    nc.vector.reciprocal(rec[:], pt[:, DH : DH + 1])
                    nc.vector.tensor_scalar(
                        out=hcat[nb][:, h * DH : (h + 1) * DH],
                        in0=pt[:, 0:DH],
                        scalar1=rec[:],
                        scalar2=None,
                        op0=OP.mult,
                    )

            if dbg and l == 0:
                for nb in range(NB):
                    nc.sync.dma_start(
                        dbg["d_hcat0"][nb * P : (nb + 1) * P, :], hcat[nb][:]
                    )
            # residual + layernorm
            for nb in range(NB):
                x = work.tile([P, D], f32, tag="x")
                nc.vector.tensor_add(x[:], h0[nb][:], hcat[nb][:])
                stats = small.tile([P, 6], f32, tag="stats")
                nc.vector.bn_stats(stats[:], x[:])
                mv = small.tile([P, 2], f32, tag="mv")
                nc.vector.bn_aggr(mv[:], stats[:])
                std = small.tile([P, 1], f32, tag="std")
                nc.scalar.activation(
                    out=std[:], in_=mv[:, 1:2], func=AF.Sqrt, bias=eps_t[:], scale=1.0
                )
                rstd = small.tile([P, 1], f32, tag="rstd")
                nc.vector.reciprocal(rstd[:], std[:])
                y = work.tile([P, D], f32, tag="y")
                nc.vector.tensor_scalar(
                    out=y[:],
                    in0=x[:],
                    scalar1=mv[:, 0:1],
                    scalar2=rstd[:],
                    op0=OP.subtract,
                    op1=OP.mult,
                )
                y2 = work.tile([P, D], f32, tag="y2")
                nc.vector.tensor_mul(y2[:], y[:], ln_s[l][:])
                nc.vector.tensor_add(h0[nb][:], y2[:], ln_b[l][:])
            if l + 1 < NLAYERS:
                build_hT()

        for nb in range(NB):
            nc.sync.dma_start(hout[nb * P : (nb + 1) * P, :], h0[nb][:])

    nc.compile()
    return nc


def _get_program():
    if "nc" not in _CACHE:
        _CACHE["nc"] = _build_program()
    return _CACHE["nc"]


def _host_prep(cfg_adj, cfg_nodes, emb, W, a_src, a_dst, ln_scale, ln_bias):
    cfg_adj = np.asarray(cfg_adj)
    eye = np.eye(N, dtype=cfg_adj.dtype)
    adj = np.minimum(cfg_adj + eye[None], 1)
    maskT = np.ascontiguousarray(adj.transpose(0, 2, 1)).astype(np.float32)
    emb16 = np.asarray(emb, np.float32) / np.float32(L)
    W = np.asarray(W, np.float32)
    wflat = W.transpose(0, 2, 1, 3).reshape(NLAYERS, D, H * DH)
    wsd = np.zeros((NLAYERS, D, 8), np.float32)
    for l in range(NLAYERS):
        for h in range(H):
            wsd[l, :, 2 * h] = W[l, h] @ np.asarray(a_src, np.float32)[l, h]
            wsd[l, :, 2 * h + 1] = W[l, h] @ np.asarray(a_dst, np.float32)[l, h]
    wlp = np.concatenate([wflat, wsd], axis=2).reshape(NLAYERS, 2, P, D + 8)
    wlp = np.ascontiguousarray(wlp)
    # wsrcb[l, h, k, d, p] = (W[l,h] @ a_src[l,h])[k*128+d] for every p
    wsrcb = np.ascontiguousarray(
        np.broadcast_to(
            wsd[:, :, 0::2].transpose(0, 2, 1).reshape(NLAYERS, H, 2, P, 1),
            (NLAYERS, H, 2, P, P),
        )
    )
    lnw = np.ascontiguousarray(
        np.stack([np.asarray(ln_scale, np.float32), np.asarray(ln_bias, np.float32)], 1)
    )
    # gidx[b, p, nb, l] = cfg_nodes[b, nb*128 + p, l]
    nodes = np.asarray(cfg_nodes, np.int32)
    gidx = np.ascontiguousarray(
        nodes.reshape(B, NB, P, L).transpose(0, 2, 1, 3)
    )
    return maskT, gidx, emb16, wlp, lnw, wsrcb


def _get_executor():
    """Compile (once) a shard_map'd jit over 8 cores running the Bass NEFF.

    Mirrors concourse.bass2jax.run_bass_via_pjrt's multi-core path, but keeps
    the jitted callable around so repeat calls / benchmarking skip recompiles.
    """
    if "exec" in _CACHE:
        return _CACHE["exec"]
    import jax
    from jax.sharding import Mesh, PartitionSpec
    from concourse import bass2jax, mybir

    try:
        from jax.experimental.shard_map import shard_map
    except ImportError:
        from jax.shard_map import shard_map  # newer jax

    bass2jax.install_neuronx_cc_hook()
    nc = _get_program()
    partition_name = nc.partition_id_tensor.name if nc.partition_id_tensor else None

    in_names, out_names, out_avals, zero_outs = [], [], [], []
    for alloc in nc.m.functions[0].allocations:
        if not isinstance(alloc, mybir.MemoryLocationSet):
            continue
        name = alloc.memorylocations[0].name
        if alloc.kind == "ExternalInput":
            if name != partition_name:
                in_names.append(name)
        elif alloc.kind == "ExternalOutput":
            out_names.append(name)
            shape = tuple(alloc.tensor_shape)
            dt = mybir.dt.np(alloc.dtype)
            out_avals.append(jax.core.ShapedArray(shape, dt))
            zero_outs.append(np.zeros(shape, dt))
    n_params, n_outs = len(in_names), len(out_names)
    all_names = in_names + out_names
    if partition_name is not None:
        all_names = all_names + [partition_name]

    def _body(*args):
        operands = list(args)
        if partition_name is not None:
            operands.append(bass2jax.partition_id_tensor())
        outs = bass2jax._bass_exec_p.bind(
            *operands,
            out_avals=tuple(out_avals),
            in_names=tuple(all_names),
            out_names=tuple(out_names),
            lowering_input_output_aliases=(),
            sim_require_finite=True,
            sim_require_nnan=True,
            nc=nc,
        )
        return tuple(outs)

    devices = jax.devices()[:B]
    mesh = Mesh(np.asarray(devices), ("core",))
    fn = jax.jit(
        shard_map(
            _body,
            mesh=mesh,
            in_specs=(PartitionSpec("core"),) * (n_params + n_outs),
            out_specs=(PartitionSpec("core"),) * n_outs,
            check_rep=False,
        ),
        keep_unused=True,
    )
    ex = {
        "fn": fn,
        "mesh": mesh,
        "in_names": in_names,
        "out_names": out_names,
        "zero_outs": zero_outs,
        "n_params": n_params,
    }
    _CACHE["exec"] = ex
    return ex


def _concat_inputs(in_maps, ex):
    out = []
    for name in ex["in_names"]:
        out.append(np.concatenate([np.asarray(m[name]) for m in in_maps], axis=0))
    for z in ex["zero_outs"]:
        out.append(np.zeros((B * z.shape[0], *z.shape[1:]), z.dtype))
    return out


def _make_in_maps(cfg_adj, cfg_nodes, emb, W, a_src, a_dst, ln_scale, ln_bias):
    maskT, gidx, emb16, wlp, lnw, wsrcb = _host_prep(
        cfg_adj, cfg_nodes, emb, W, a_src, a_dst, ln_scale, ln_bias
    )
    return [
        {
            "adjtf": maskT[b],
            "gidx": gidx[b],
            "embs": emb16,
            "wl": wlp,
            "lnw": lnw,
            "wsrcb": wsrcb,
        }
        for b in range(B)
    ]


def kernel(cfg_adj, cfg_nodes, cfg_len, emb, W, a_src, a_dst, ln_scale, ln_bias):
    ex = _get_executor()
    in_maps = _make_in_maps(cfg_adj, cfg_nodes, emb, W, a_src, a_dst, ln_scale, ln_bias)
    args = _concat_inputs(in_maps, ex)
    outs = ex["fn"](*args)
    h = np.asarray(outs[ex["out_names"].index("hout")]).reshape(B, N, D)
    return h, np.asarray(cfg_len, np.int32)


def benchmark(np_inputs, iters=30, warmup=3):
    """Amortized per-execute wall time (ns) with device-resident inputs."""
    import time
    import jax
    from jax.sharding import NamedSharding, PartitionSpec

    ex = _get_executor()
    in_maps = _make_in_maps(
        np_inputs["cfg_adj"], np_inputs["cfg_nodes"], np_inputs["emb"],
        np_inputs["W"], np_inputs["a_src"], np_inputs["a_dst"],
        np_inputs["ln_scale"], np_inputs["ln_bias"],
    )
    args = _concat_inputs(in_maps, ex)
    sharding = NamedSharding(ex["mesh"], PartitionSpec("core"))
    dev_args = [jax.device_put(a, sharding) for a in args]
    fn = ex["fn"]
    for _ in range(warmup):
        r = fn(*dev_args)
    jax.block_until_ready(r)
    t0 = time.perf_counter()
    for _ in range(iters):
        r = fn(*dev_args)
    jax.block_until_ready(r)
    dt = (time.perf_counter() - t0) / iters
    return dt * 1e9
